# revision 30
# baseline (speedup 1.0000x reference)
"""Trainium2 Bass kernel for AsyncFeatureExtraction (segment_reduce).

See module docstring history: v4 introduced the padded channel grid +
step-histogram formulation; v5 is a latency pass over it:
  - one packed constant DMA instead of 14
  - one packed x DMA instead of 3
  - routing split into a rank-independent plane-building loop (deep bufs)
    and a matmul loop, so the rank DRAM roundtrip overlaps
  - grid -> DRAM -> broadcast -> all-pairs min pipelined in 4 channel groups
  - all 32 step tiles pre-built before the histogram matmuls

Math (per batch, 1 batch per core):
  * rank[n] = # earlier same-channel points, via segmented cumsum scan +
    exact matmul extraction (+0.25 guard for the int cast).
  * grid routing: grid += rankOH_c.T @ [t_hi|t_lo|occ|v] (bf16, exact
    placement; t split exactly into two bf16 planes).
  * inv_density: per channel all-pairs |t_i - t_j| over its 128-slot grid
    column; diagonal/empties killed by BIG sentinels; dw = exp(ks*ln(ivd)).
  * Z/cnt/V/ZT1 as cumulative step-histograms: one matmul per channel with
    stationary step_c[r,tau] = (t_g[r,c] <= pos[tau]); S1 = ZT1/max_pos -
    (pos/max_pos)*Z;  out = Wd2@(S1*R) + We2@(Z*R) + Wv2@(V*R) + b_lin,
    R = 1/((Z+eps)(cnt+eps)), with (tau,c)->(c,tau) via identity matmuls.
"""

import os
import numpy as np

B, N, T, C, D, CO = 8, 3072, 128, 32, 8, 64
P = 128
NCH = N // P
NSEG = 4
SEGN = N // NSEG
G = C * P
NG = 4                # channel groups for the banded pipeline
CG = C // NG          # 8 channels per group
BIG = 1e10

_cache = {}

# packed const layout (free-dim offsets in the (128, CW) const block)
_OFF = {}
_cw = 0
for _name, _w in [
    ("pos", T), ("eye", P), ("ident", P), ("crow", C), ("irow", P), ("esel", C * P),
    ("segsel", NSEG), ("chm", P), ("iota", 1), ("wd2", CO), ("we2", CO),
    ("wv2", CO), ("blin", 1), ("ks", 1), ("imp", 1), ("pmp", 1),
]:
    _OFF[_name] = (_cw, _w)
    _cw += _w
CW = _cw


def _build_nc():
    from contextlib import ExitStack

    import concourse.bass as bass
    import concourse.tile as tile
    from concourse import bacc, mybir

    f32 = mybir.dt.float32
    bf16 = mybir.dt.bfloat16
    i32 = mybir.dt.int32
    ALU = mybir.AluOpType
    ACT = mybir.ActivationFunctionType
    AX = mybir.AxisListType

    nc = bacc.Bacc(None)

    xT = nc.declare_dram_parameter("xT", [3, N], f32, isOutput=False)
    cst = nc.declare_dram_parameter("cst", [P, CW], f32, isOutput=False)
    out_ext = nc.declare_dram_parameter("out", [CO, T], f32, isOutput=True)

    rank_d = nc.dram_tensor("rank_d", [N, 1], i32)
    grid_d = nc.dram_tensor("grid_d", [G, 1], f32)

    def dram_ap(handle, offset, pattern):
        return bass.AP(handle[:].tensor, offset, pattern)

    with tile.TileContext(nc) as tc, ExitStack() as ctx:
        const = ctx.enter_context(tc.tile_pool(name="const", bufs=1))
        pp = ctx.enter_context(tc.tile_pool(name="perpoint", bufs=1))
        rk = ctx.enter_context(tc.tile_pool(name="rank", bufs=1))
        xcp = ctx.enter_context(tc.tile_pool(name="xcp", bufs=1))
        rkp = ctx.enter_context(tc.tile_pool(name="rkp", bufs=6))
        gr = ctx.enter_context(tc.tile_pool(name="grid", bufs=1))
        sgp = ctx.enter_context(tc.tile_pool(name="sgp", bufs=1))
        band = ctx.enter_context(tc.tile_pool(name="band", bufs=6))
        stp = ctx.enter_context(tc.tile_pool(name="step", bufs=1))
        psum = ctx.enter_context(tc.tile_pool(name="psum", bufs=1, space="PSUM"))
        sb = ctx.enter_context(tc.tile_pool(name="stageD", bufs=1))

        # ---- small data DMAs first (don't queue behind the 2.5MB consts) --
        pv = pp.tile([P, 3, NCH], f32)
        nc.sync.dma_start(pv[:], dram_ap(xT, 0, [[NCH, P], [N, 3], [1, NCH]]))
        f_seg = rk.tile([P, SEGN], f32)
        for s in range(NSEG):
            nc.sync.dma_start(
                f_seg[32 * s : 32 * s + 32, :],
                xT[0][SEGN * s : SEGN * (s + 1)][None, :].to_broadcast([32, SEGN]),
            )

        # ---- packed constants: one DMA ----
        cst_t = const.tile([P, CW], f32)
        nc.sync.dma_start(cst_t[:], cst[:])

        def cslice(name, rows=P):
            o, w = _OFF[name]
            return cst_t[0:rows, o : o + w]

        pos_t = cslice("pos")
        eye_t = cslice("eye")
        id_t = cslice("ident")
        crow_t = cslice("crow")
        irow_t = cslice("irow")
        esel_t = cslice("esel", C)
        segsel_t = cslice("segsel")
        chm_t = cslice("chm")
        iota_c = cslice("iota")
        wd2_t = cslice("wd2", C)
        we2_t = cslice("we2", C)
        wv2_t = cslice("wv2", C)
        blin_c = cslice("blin", CO)
        ks_c = cslice("ks")
        imp_c = cslice("imp")
        pmp_c = cslice("pmp")

        id_b = const.tile([P, P], bf16)
        nc.vector.tensor_copy(id_b[:], id_t)

        # (pv DMA issued before the big const DMA; see top)
        f_t = pv[:, 0, :]
        v_t = pv[:, 1, :]
        t_t = pv[:, 2, :]

        thi_t = pp.tile([P, NCH], bf16)
        nc.vector.tensor_copy(thi_t[:], t_t)
        thi_f = pp.tile([P, NCH], f32)
        nc.vector.tensor_copy(thi_f[:], thi_t[:])
        tlo_t = pp.tile([P, NCH], f32)
        nc.vector.tensor_tensor(tlo_t[:], t_t, thi_f[:], op=ALU.subtract)

        # ---- stage R: per-channel ranks via segmented scan ----
        # (f_seg DMAs issued before the big const DMA; see top)
        oh_seg = rk.tile([P, SEGN], f32)
        nc.vector.tensor_scalar(oh_seg[:], f_seg[:], iota_c, None, ALU.is_equal)
        zseg = rk.tile([P, SEGN], f32)
        nc.vector.memset(zseg[:], 0.0)
        csum = rk.tile([P, SEGN], f32)
        nc.vector.tensor_tensor_scan(
            csum[:], oh_seg[:], zseg[:], 0.0, op0=ALU.add, op1=ALU.add
        )
        totals = rk.tile([P, 1], f32)
        nc.vector.tensor_copy(totals[:], csum[:, SEGN - 1 : SEGN])
        a_p = psum.tile([P, 1], f32, tag="scratch")
        nc.tensor.matmul(a_p[:], lhsT=chm_t, rhs=totals[:], start=True, stop=True)
        a_s = rk.tile([P, 1], f32)
        nc.vector.tensor_scalar(a_s[:], a_p[:], -0.75, None, ALU.add)
        csum2 = rk.tile([P, SEGN], f32)
        nc.vector.tensor_scalar(csum2[:], csum[:], a_s[:, 0:1], None, ALU.add)
        maskg = rk.tile([P, SEGN], f32)
        nc.vector.tensor_tensor(maskg[:], csum2[:], oh_seg[:], op=ALU.mult)
        g_p = psum.tile([NSEG, SEGN], f32, tag="scratch")
        nc.tensor.matmul(
            g_p[:, 0:512], lhsT=segsel_t, rhs=maskg[:, 0:512], start=True, stop=True
        )
        nc.tensor.matmul(
            g_p[:, 512:SEGN], lhsT=segsel_t, rhs=maskg[:, 512:SEGN],
            start=True, stop=True,
        )
        g_i = rk.tile([NSEG, SEGN], i32)
        nc.vector.tensor_copy(g_i[:], g_p[:])
        nc.sync.dma_start(dram_ap(rank_d, 0, [[SEGN, NSEG], [1, SEGN]]), g_i[:])
        rank_i = pp.tile([P, NCH], i32)
        nc.sync.dma_start(rank_i[:], dram_ap(rank_d, 0, [[NCH, P], [1, NCH]]))
        rank_t = pp.tile([P, NCH], f32)
        nc.vector.tensor_copy(rank_t[:], rank_i[:])

        # ---- routing loop 1 (rank-independent): value planes per chunk ----
        xcs = []
        for ch in range(NCH):
            xc = xcp.tile([P, 4 * C], bf16, tag=f"xc{ch}")
            oh_sl = xc[:, 2 * C : 3 * C]
            nc.vector.tensor_scalar(
                oh_sl, crow_t, f_t[:, ch : ch + 1], None, ALU.is_equal
            )
            nc.vector.tensor_scalar(
                xc[:, 0:C], oh_sl, thi_f[:, ch : ch + 1], None, ALU.mult
            )
            nc.vector.tensor_scalar(
                xc[:, C : 2 * C], oh_sl, tlo_t[:, ch : ch + 1], None, ALU.mult
            )
            nc.vector.tensor_scalar(
                xc[:, 3 * C : 4 * C], oh_sl, v_t[:, ch : ch + 1], None, ALU.mult
            )
            xcs.append(xc)

        # ---- routing loop 2: rank one-hots + accumulating matmuls ----
        grid_p = psum.tile([P, 4 * C], f32, tag="scratch")
        for ch in range(NCH):
            rkoh = rkp.tile([P, P], bf16, tag="rkoh")
            nc.vector.tensor_scalar(
                rkoh[:], irow_t, rank_t[:, ch : ch + 1], None, ALU.is_equal
            )
            nc.tensor.matmul(
                grid_p[:], lhsT=rkoh[:], rhs=xcs[ch][:],
                start=(ch == 0), stop=(ch == NCH - 1),
            )

        t_g = gr.tile([P, C], f32)
        nc.vector.tensor_copy(t_g[:], grid_p[:, 0:C])
        nc.vector.tensor_tensor(t_g[:], t_g[:], grid_p[:, C : 2 * C], op=ALU.add)
        occ_g = gr.tile([P, C], f32)
        nc.vector.tensor_copy(occ_g[:], grid_p[:, 2 * C : 3 * C])
        v_g = gr.tile([P, C], f32)
        nc.vector.tensor_copy(v_g[:], grid_p[:, 3 * C : 4 * C])

        s_g = gr.tile([P, C], f32)
        nc.vector.tensor_scalar(s_g[:], occ_g[:], BIG, -BIG, ALU.mult, op1=ALU.add)
        nc.vector.tensor_tensor(s_g[:], s_g[:], t_g[:], op=ALU.add)
        neg_s = gr.tile([P, C], f32)
        nc.vector.tensor_scalar(neg_s[:], s_g[:], -1.0, None, ALU.mult)

        # ---- pre-build all step tiles (only needs t_g) ----
        steps = []
        for ch in range(C):
            step = stp.tile([P, P], bf16, tag=f"st{ch}")
            nc.vector.tensor_scalar(
                step[:], pos_t, t_g[:, ch : ch + 1], None, ALU.is_ge
            )
            steps.append(step)

        # ---- stage B: on-chip broadcast + all-pairs min (no DRAM trip) ----
        # sT[ch, r] = s_g[r, ch] via identity matmul, then per channel a
        # 1-partition ones-matmul broadcasts row ch into PSUM for ScalarE.
        st_p = psum.tile([C, P], f32, tag="tp0")
        nc.tensor.matmul(st_p[:], lhsT=s_g[:], rhs=id_t, start=True, stop=True)
        st_s = gr.tile([C, P], f32)
        nc.vector.tensor_copy(st_s[:], st_p[:])

        ivd_g = gr.tile([P, C], f32)
        for ch in range(C):
            sgb = psum.tile([P, P], f32, tag=f"sgb{ch % 2}")
            nc.tensor.matmul(
                sgb[:], lhsT=esel_t[:, ch * P : (ch + 1) * P], rhs=st_s[:],
                start=True, stop=True,
            )
            dbuf = band.tile([P, P], f32, tag="dbuf")
            nc.scalar.activation(
                dbuf[:], sgb[:], ACT.Abs, bias=neg_s[:, ch : ch + 1], scale=1.0
            )
            nc.vector.tensor_tensor(dbuf[:], dbuf[:], eye_t, op=ALU.add)
            nc.vector.tensor_reduce(
                ivd_g[:, ch : ch + 1], dbuf[:], axis=AX.X, op=ALU.min
            )
        nc.vector.tensor_scalar(ivd_g[:], ivd_g[:], 2.0**-11, None, ALU.max)

        dw_g = gr.tile([P, C], f32)
        nc.scalar.activation(dw_g[:], ivd_g[:], ACT.Ln)
        nc.scalar.activation(dw_g[:], dw_g[:], ACT.Exp, scale=ks_c)

        # ---- stage H: weight planes + per-channel histogram matmuls ----
        w2f = gr.tile([P, C], f32)
        nc.vector.tensor_tensor(w2f[:], occ_g[:], dw_g[:], op=ALU.mult)
        w3f = gr.tile([P, C], f32)
        nc.vector.tensor_tensor(w3f[:], w2f[:], v_g[:], op=ALU.mult)
        w2t = gr.tile([P, C], f32)
        nc.vector.tensor_tensor(w2t[:], w2f[:], t_g[:], op=ALU.mult)
        wstack = gr.tile([P, C, 4], bf16)
        nc.vector.tensor_copy(wstack[:, :, 0:1], occ_g[:, :, None])
        nc.vector.tensor_copy(wstack[:, :, 1:2], w2f[:, :, None])
        nc.vector.tensor_copy(wstack[:, :, 2:3], w3f[:, :, None])
        nc.vector.tensor_copy(wstack[:, :, 3:4], w2t[:, :, None])

        hist_p = psum.tile([P, C, 4], f32, tag="hist")
        for ch in range(C):
            nc.tensor.matmul(
                hist_p[:, ch, :], lhsT=steps[ch][:], rhs=wstack[:, ch, :],
                start=True, stop=True,
            )

        # ---- stage D: combine (tau on partitions) ----
        cnt_v = hist_p[:, :, 0]
        z_v = hist_p[:, :, 1]
        v_v = hist_p[:, :, 2]
        zt1_v = hist_p[:, :, 3]

        r_t = sb.tile([P, C], f32)
        ce_t = sb.tile([P, C], f32)
        nc.vector.tensor_scalar(r_t[:], z_v, 1e-10, None, ALU.add)
        nc.vector.tensor_scalar(ce_t[:], cnt_v, 1e-10, None, ALU.add)
        nc.vector.tensor_tensor(r_t[:], r_t[:], ce_t[:], op=ALU.mult)
        nc.vector.reciprocal(r_t[:], r_t[:])

        s1_t = sb.tile([P, C], f32)
        nc.vector.tensor_scalar(s1_t[:], zt1_v, imp_c, None, ALU.mult)
        zp_t = sb.tile([P, C], f32)
        nc.vector.tensor_scalar(zp_t[:], z_v, pmp_c, None, ALU.mult)
        nc.vector.tensor_tensor(s1_t[:], s1_t[:], zp_t[:], op=ALU.subtract)

        s1r = sb.tile([P, C], f32)
        nc.vector.tensor_tensor(s1r[:], s1_t[:], r_t[:], op=ALU.mult)
        zr = sb.tile([P, C], f32)
        nc.vector.tensor_tensor(zr[:], z_v, r_t[:], op=ALU.mult)
        vr = sb.tile([P, C], f32)
        nc.vector.tensor_tensor(vr[:], v_v, r_t[:], op=ALU.mult)

        outs = []
        for k, src in enumerate((s1r, zr, vr)):
            src_b = sb.tile([P, C], bf16, tag=f"sb{k}")
            nc.vector.tensor_copy(src_b[:], src[:])
            tp = psum.tile([C, P], f32, tag=f"tp{k}")
            nc.tensor.matmul(tp[:], lhsT=src_b[:], rhs=id_b[:], start=True, stop=True)
            sbuf_t = sb.tile([C, P], f32, tag=f"tr{k}")
            nc.vector.tensor_copy(sbuf_t[:], tp[:])
            outs.append(sbuf_t)

        out_p = psum.tile([CO, T], f32, tag="scratch")
        nc.tensor.matmul(out_p[:], lhsT=wd2_t, rhs=outs[0][:], start=True, stop=False)
        nc.tensor.matmul(out_p[:], lhsT=we2_t, rhs=outs[1][:], start=False, stop=False)
        nc.tensor.matmul(out_p[:], lhsT=wv2_t, rhs=outs[2][:], start=False, stop=True)

        out_t = sb.tile([CO, T], f32)
        nc.vector.tensor_scalar(out_t[:], out_p[:], blin_c, None, ALU.add)
        nc.sync.dma_start(out_ext[:], out_t[:])

    nc.compile()
    return nc


def _prep_inputs(x, out_positions, W_dist, b_dist, emb, W_vals, b_vals, W_lin, b_lin, kernel_scale):
    x = np.asarray(x, np.float32)
    pos = np.asarray(out_positions, np.float32)
    max_pos = float(pos.max())
    Wl = np.asarray(W_lin, np.float32).reshape(CO, C, D)
    emb2 = np.asarray(emb, np.float32)[:C] + np.asarray(b_dist, np.float32) + np.asarray(
        b_vals, np.float32
    )
    wd2 = (Wl * np.asarray(W_dist, np.float32)).sum(-1).T
    we2 = np.einsum("ocd,cd->oc", Wl, emb2).T
    wv2 = (Wl * np.asarray(W_vals, np.float32)).sum(-1).T

    q = np.arange(P)
    seg_sel = ((q // C)[:, None] == np.arange(NSEG)[None, :]).astype(np.float32)
    chm_m = (
        ((q % C)[:, None] == (q % C)[None, :])
        & ((q // C)[:, None] < (q // C)[None, :])
    ).astype(np.float32)

    cst = np.zeros((P, CW), np.float32)

    def put(name, arr, rows=P):
        o, w = _OFF[name]
        cst[0:rows, o : o + w] = arr

    put("pos", np.tile(pos[None, :], (P, 1)))
    put("eye", np.eye(P, dtype=np.float32) * BIG)
    put("ident", np.eye(P, dtype=np.float32))
    put("crow", np.tile(np.arange(C, dtype=np.float32), (P, 1)))
    put("irow", np.tile(np.arange(P, dtype=np.float32), (P, 1)))
    put("esel", np.kron(np.eye(C, dtype=np.float32), np.ones((1, P), np.float32)), C)
    put("segsel", seg_sel)
    put("chm", chm_m)
    put("iota", (q % C).astype(np.float32)[:, None])
    put("wd2", wd2.astype(np.float32), C)
    put("we2", we2.astype(np.float32), C)
    put("wv2", wv2.astype(np.float32), C)
    put("blin", np.asarray(b_lin, np.float32)[:, None], CO)
    put("ks", np.full((P, 1), float(kernel_scale), np.float32))
    put("imp", np.full((P, 1), 1.0 / max_pos, np.float32))
    put("pmp", (pos / max_pos)[:, None])

    in_maps = []
    for b in range(B):
        in_maps.append({"xT": np.ascontiguousarray(x[b].T), "cst": cst})
    return in_maps


def kernel(**inputs) -> np.ndarray:
    from concourse.bass_utils import run_bass_kernel_spmd

    if "nc" not in _cache:
        _cache["nc"] = _build_nc()
    nc = _cache["nc"]

    in_maps = _prep_inputs(**inputs)
    res = run_bass_kernel_spmd(
        nc, in_maps, core_ids=list(range(B)),
        trace=bool(int(os.environ.get("KERNEL_TRACE", "0"))),
    )
    if res.exec_time_ns is not None:
        _cache["exec_time_ns"] = res.exec_time_ns
        _cache["last_result"] = res
    out = np.stack([res.results[i]["out"] for i in range(B)]).astype(np.float32)
    return out
